# revision 1
# baseline (speedup 1.0000x reference)
"""PointNet feature-propagation block on 8 Trainium2 NeuronCores.

Data-parallel over the batch dim: 16 batches -> 2 per core.
Per batch on-device pipeline:
  1. G = feat @ W1^T (+b1) per sparse point (PE); rows [512 x fp16 | xs coords
     as 3 x fp32 | pad] (1280 B) stored to HBM.
  2. Scores S = -d2 via augmented K=5 matmul (PE):
       lhsT rows [2x,2y,2z,|xd|^2,1], rhs rows [x,y,z,-1,-|xs|^2].
  3. Top-8 candidates per dense point: DVE max + max_index; keep 4.
  4. dma_gather of the 4 candidate G-rows (with coords) per dense point.
  5. Exact d2 recomputed from gathered coords; the worst of the 4 candidates
     is excluded by a zero weight; w_m = (1/(sqrt(d2_m)+eps)) normalized.
  6. h = relu(sum_m w_m * g_m)  (fused DVE ops, per-partition scalars).
  7. h^T via XBAR DMA transposes (HWDGE).
  8. out = h @ W2^T (+b2) (PE), stored as (dense, ch) fp32.
"""
import numpy as np

import concourse.bass as bass
import concourse.tile as tile
import concourse.mybir as mybir
from concourse import bacc
from concourse.bass_utils import run_bass_kernel_spmd

B, N1, N2 = 16, 1024, 4096
C_IN, C_OUT = 512, 512
NCAND = 4                    # candidates gathered per dense point
EPS = 1e-10
N_CORES = 8
BPC = B // N_CORES           # batches per core
NCH = N2 // 128              # dense chunks per batch (32)
NR = N1 // 128               # sparse chunks (8)
NQI = C_IN // 128            # input-channel chunks (4)
NQO = C_OUT // 128           # output-channel chunks (4)
JQ = 8                       # dense chunks per gather group
NQUARTER = NCH // JQ         # 4
GROW = 640                   # G row length in fp16 elems (512 feat + 6 coord + pad)

F32 = mybir.dt.float32
F16 = mybir.dt.float16
U16 = mybir.dt.uint16
I16 = mybir.dt.int16
Alu = mybir.AluOpType
Act = mybir.ActivationFunctionType
AxX = mybir.AxisListType.X

# dev-only ablation switches (empty for production)
_ABLATE = set()


def _emit_front(nc, tc, pools, aps, b, include_b1, include_b2):
    (sb, gpool, hpool, htpool, opool, wpool, fpool, wsump, psum_s, psum_g, psum_o) = pools
    (xdT, xsT, featT, xd_pc, xs_pc, w1T_sb, w2T_sb, b1row_sb, b2row_sb,
     ones_sb, ones16_sb, identq_sb, out_ap, g_dram) = aps

    # ---- Stage 1: G rows = [feat @ W1^T (+b1) | xs coords | pad] -----------
    featT_sb = sb.tile([128, NQI, N1], F32, tag="featT")
    for q in range(NQI):
        nc.sync.dma_start(featT_sb[:, q, :], featT[b, q * 128:(q + 1) * 128, :])
    xs_pc_sb = sb.tile([128, NR, 3], F32, tag="xs_pc")
    nc.sync.dma_start(xs_pc_sb[:], xs_pc[b])

    g_sb = sb.tile([128, NR, GROW], F16, tag="g_sb")
    for r in range(NR):
        pg = psum_g.tile([128, C_OUT], F32)
        for q in range(NQI):
            nc.tensor.matmul(
                pg[:], featT_sb[:, q, r * 128:(r + 1) * 128], w1T_sb[:, q, :],
                start=(q == 0), stop=(q == NQI - 1 and not include_b1))
        if include_b1:
            nc.tensor.matmul(pg[:], ones_sb[0:1, :], b1row_sb[:],
                             start=False, stop=True)
        nc.scalar.activation(g_sb[:, r, 0:C_OUT], pg[:], Act.Copy)
    # coords + zero pad
    nc.vector.tensor_copy(g_sb[:, :, C_OUT:C_OUT + 6].bitcast(F32), xs_pc_sb[:])
    nc.vector.memset(g_sb[:, :, C_OUT + 6:GROW], 0.0)
    # store to HBM with row-major (1024, GROW) layout: row r*128+p
    nc.sync.dma_start(g_dram[b].rearrange("(r p) e -> p r e", p=128), g_sb[:])

    # ---- Stage 2: scores + top-8 candidates -------------------------------
    xdT_sb = sb.tile([5, N2], F32, tag="xdT")
    nc.sync.dma_start(xdT_sb[:], xdT[b])
    xsT_sb = sb.tile([5, N1], F32, tag="xsT")
    nc.sync.dma_start(xsT_sb[:], xsT[b])
    xd_pc_sb = fpool.tile([128, NCH, 3], F32, tag="xd_pc")
    nc.sync.dma_start(xd_pc_sb[:], xd_pc[b])

    m8 = fpool.tile([128, NCH, 8], F32, tag="m8")
    i8 = fpool.tile([128, NCH, 8], U16, tag="i8")
    for c in range(NCH):
        ps = psum_s.tile([128, N1], F32)
        lhs = xdT_sb[:, c * 128:(c + 1) * 128]
        nc.tensor.matmul(ps[:, 0:512], lhs, xsT_sb[:, 0:512], start=True, stop=True)
        nc.tensor.matmul(ps[:, 512:1024], lhs, xsT_sb[:, 512:1024], start=True, stop=True)
        if "topk" not in _ABLATE:
            nc.vector.max(m8[:, c, :], ps[:])
            nc.vector.max_index(i8[:, c, :], m8[:, c, :], ps[:])
        else:
            nc.vector.memset(m8[:, c, :], 0.5)
            nc.vector.memset(i8[:, c, :], 1)
    return m8, i8, xd_pc_sb


def _emit_back(nc, tc, pools, aps, b, include_b1, include_b2, front):
    (sb, gpool, hpool, htpool, opool, wpool, fpool, wsump, psum_s, psum_g, psum_o) = pools
    (xdT, xsT, featT, xd_pc, xs_pc, w1T_sb, w2T_sb, b1row_sb, b2row_sb,
     ones_sb, ones16_sb, identq_sb, out_ap, g_dram) = aps
    m8, i8, xd_pc_sb = front

    # ---- Stage 3: candidate index lists in wrapped int16 layout ------------
    idxw = sb.tile([128, NCAND, N2 // 16], I16, tag="idxw")
    if "scatter" in _ABLATE:
        nc.vector.memset(idxw[:], 0)
    for m in range(NCAND) if "scatter" not in _ABLATE else []:
        # dst[p%16, c*8 + p//16] = i8[p, c, m]
        dstv = idxw[0:16, m, :].rearrange("q (c pp) -> q c pp", pp=8)
        for pp in range(8):
            nc.sync.dma_start(dstv[:, :, pp],
                              i8[16 * pp:16 * (pp + 1), :, m].bitcast(I16))
    for r in range(1, 8):
        nc.sync.dma_start(idxw[16 * r:16 * (r + 1), :, :], idxw[0:16, :, :])

    # ---- Stages 4-6: gather, exact d2, weights, weighted sum (per quarter) -
    h_sb = hpool.tile([128, NCH, C_OUT], F16, tag="h_sb")
    if "wsum" in _ABLATE:
        nc.vector.memset(h_sb[:], 1.0)
    for Q in range(NQUARTER):
        js = slice(Q * JQ, (Q + 1) * JQ)
        gk = []
        for m in range(NCAND):
            t = gpool.tile([128, JQ, GROW], F16, tag=f"gk{m}")
            if "gather" in _ABLATE:
                nc.scalar.activation(t[:], t[:], Act.Copy) if False else nc.vector.memset(t[:], 1.0)
            else:
                nc.gpsimd.dma_gather(
                    out_ap=t[:], in_ap=g_dram[b],
                    idxs_ap=idxw[:, m, Q * (JQ * 128 // 16):(Q + 1) * (JQ * 128 // 16)],
                    num_idxs=JQ * 128, num_idxs_reg=JQ * 128, elem_size=GROW)
            gk.append(t)

        # exact d2 for each candidate from gathered coords
        d2q = wpool.tile([128, JQ, NCAND], F32, tag="d2q")
        for m in range(NCAND):
            cview = gk[m][:, :, C_OUT:C_OUT + 6].bitcast(F32)
            diff = wpool.tile([128, JQ, 3], F32, tag="cdiff")
            nc.vector.tensor_tensor(diff[:], xd_pc_sb[:, js, :], cview, Alu.subtract)
            sq = wpool.tile([128, JQ, 3], F32, tag="csq")
            nc.vector.tensor_tensor(sq[:], diff[:], diff[:], Alu.mult)
            nc.vector.tensor_reduce(d2q[:, :, m:m + 1], sq[:], AxX, Alu.add)

        # weights: u = 1/(sqrt(d2)+eps); exclude the max-d2 candidate; norm
        dmax = wpool.tile([128, JQ, 1], F32, tag="dmax")
        nc.vector.tensor_reduce(dmax[:], d2q[:], AxX, Alu.max)
        keep = wpool.tile([128, JQ, NCAND], F32, tag="keep")
        # keep = (d2 < dmax) -> excluded candidate gets 0
        nc.vector.tensor_tensor(keep[:], d2q[:],
                                dmax[:].broadcast_to([128, JQ, NCAND]), Alu.is_lt)
        dist = wpool.tile([128, JQ, NCAND], F32, tag="cdist")
        nc.scalar.activation(dist[:], d2q[:], Act.Sqrt, bias=EPS * EPS)
        u = wpool.tile([128, JQ, NCAND], F32, tag="cu")
        nc.vector.reciprocal(u[:], dist[:])
        u0 = wpool.tile([128, JQ, NCAND], F32, tag="cu0")
        nc.vector.tensor_tensor(u0[:], u[:], keep[:], Alu.mult)
        usum = wpool.tile([128, JQ, 1], F32, tag="cusum")
        nc.vector.tensor_reduce(usum[:], u0[:], AxX, Alu.add)
        sf = wpool.tile([128, JQ, 1], F32, tag="csf")
        nc.vector.reciprocal(sf[:], usum[:])
        w = wpool.tile([128, JQ, NCAND], F32, tag="cw")
        nc.vector.tensor_tensor(w[:], u0[:],
                                sf[:].broadcast_to([128, JQ, NCAND]), Alu.mult)

        for jj in range(JQ) if "wsum" not in _ABLATE else []:
            # t_m = g_m * w_m (tensor_scalar, 4x-capable), then a tree of adds
            tm = []
            for m in range(NCAND):
                t = wsump.tile([128, C_OUT], F16, tag="wsum")
                nc.vector.tensor_scalar(t[:], gk[m][:, jj, 0:C_OUT],
                                        scalar1=w[:, jj, m:m + 1],
                                        op0=Alu.mult, scalar2=1.0, op1=Alu.mult)
                tm.append(t)
            a0 = wsump.tile([128, C_OUT], F16, tag="wsum")
            nc.vector.tensor_tensor(a0[:], tm[0][:], tm[1][:], Alu.add)
            a1 = wsump.tile([128, C_OUT], F16, tag="wsum")
            nc.vector.tensor_tensor(a1[:], tm[2][:], tm[3][:], Alu.add)
            a2 = wsump.tile([128, C_OUT], F16, tag="wsum")
            nc.vector.tensor_tensor(a2[:], a0[:], a1[:], Alu.add)
            nc.vector.tensor_scalar(h_sb[:, Q * JQ + jj, :], a2[:], scalar1=0.0,
                                    op0=Alu.max, scalar2=1.0, op1=Alu.mult)

    # ---- Stage 7: h^T via XBAR DMA transposes (HWDGE) ----------------------
    hT = htpool.tile([128, NQO, N2], F16, tag="hT")
    if "transpose" in _ABLATE:
        nc.vector.memset(hT[:], 1.0)
    else:
        for c in range(NCH):
            nc.sync.dma_start_transpose(hT[:, :, c * 128:(c + 1) * 128],
                                        h_sb[:, c, :])

    # ---- Stage 8: layer 2 --------------------------------------------------
    for c in range(NCH):
        po = psum_o.tile([128, C_OUT], F32)
        for q in range(NQO):
            nc.tensor.matmul(
                po[:], hT[:, q, c * 128:(c + 1) * 128], w2T_sb[:, q, :],
                start=(q == 0), stop=(q == NQO - 1 and not include_b2))
        if include_b2:
            nc.tensor.matmul(po[:], ones16_sb[0:1, :], b2row_sb[:],
                             start=False, stop=True)
        osb = opool.tile([128, C_OUT], F32, tag="osb")
        nc.scalar.activation(osb[:], po[:], Act.Copy)
        nc.sync.dma_start(out_ap[b, c * 128:(c + 1) * 128, :], osb[:])


def _build(include_b1, include_b2):
    nc = bacc.Bacc("TRN2", target_bir_lowering=False, debug=False,
                   num_devices=N_CORES)

    # register EPS^2 as a const AP so it can be an ACT Sqrt bias
    # (sqrt(d2 + EPS^2) == sqrt(d2) + EPS at fp32 for all reachable d2)
    _ct = nc.alloc_sbuf_tensor("const-float32-epssq", [128, 1], F32)
    nc.gpsimd.memset(_ct.ap(), EPS * EPS)
    nc.const_aps.aps[(F32, EPS * EPS)] = _ct.ap()
    nc.all_engine_barrier()

    xdT = nc.dram_tensor("xdT", [BPC, 5, N2], F32, kind="ExternalInput").ap()
    xsT = nc.dram_tensor("xsT", [BPC, 5, N1], F32, kind="ExternalInput").ap()
    featT = nc.dram_tensor("featT", [BPC, C_IN, N1], F32, kind="ExternalInput").ap()
    xd_pc = nc.dram_tensor("xd_pc", [BPC, 128, NCH, 3], F32, kind="ExternalInput").ap()
    xs_pc = nc.dram_tensor("xs_pc", [BPC, 128, NR, 3], F32, kind="ExternalInput").ap()
    w1T = nc.dram_tensor("w1T", [C_IN, C_OUT], F32, kind="ExternalInput").ap()
    w2T = nc.dram_tensor("w2T", [C_IN, C_OUT], F16, kind="ExternalInput").ap()
    b1row = nc.dram_tensor("b1row", [1, C_OUT], F32, kind="ExternalInput").ap()
    b2row = nc.dram_tensor("b2row", [1, C_OUT], F16, kind="ExternalInput").ap()
    identq = nc.dram_tensor("identq", [128, N2 // 16], I16, kind="ExternalInput").ap()
    out_ap = nc.dram_tensor("out", [BPC, N2, C_OUT], F32, kind="ExternalOutput").ap()
    g_dram = nc.dram_tensor("gscratch", [BPC, N1, GROW], F16).ap()

    with tile.TileContext(nc) as tc:
        with (
            tc.tile_pool(name="sb", bufs=1) as sb,
            tc.tile_pool(name="gpool", bufs=1) as gpool,
            tc.tile_pool(name="hpool", bufs=1) as hpool,
            tc.tile_pool(name="htpool", bufs=1) as htpool,
            tc.tile_pool(name="opool", bufs=4) as opool,
            tc.tile_pool(name="wpool", bufs=3) as wpool,
            tc.tile_pool(name="fpool", bufs=2) as fpool,
            tc.tile_pool(name="wsump", bufs=12) as wsump,
            tc.tile_pool(name="const", bufs=1) as constp,
            tc.tile_pool(name="psum_s", bufs=3, space="PSUM") as psum_s,
            tc.tile_pool(name="psum_g", bufs=1, space="PSUM") as psum_g,
            tc.tile_pool(name="psum_o", bufs=1, space="PSUM") as psum_o,
        ):
            w1T_sb = constp.tile([128, NQI, C_OUT], F32, tag="w1T")
            for q in range(NQI):
                nc.sync.dma_start(w1T_sb[:, q, :], w1T[q * 128:(q + 1) * 128, :])
            w2T_sb = constp.tile([128, NQI, C_OUT], F16, tag="w2T")
            for q in range(NQI):
                nc.sync.dma_start(w2T_sb[:, q, :], w2T[q * 128:(q + 1) * 128, :])
            b1row_sb = constp.tile([1, C_OUT], F32, tag="b1row")
            b2row_sb = constp.tile([1, C_OUT], F16, tag="b2row")
            identq_sb = constp.tile([128, N2 // 16], I16, tag="identq")
            nc.sync.dma_start(identq_sb[:], identq[:])
            ones_sb = constp.tile([1, 128], F32, tag="ones")
            ones16_sb = constp.tile([1, 128], F16, tag="ones16")
            if include_b1:
                nc.vector.memset(ones_sb[:], 1.0)
                nc.sync.dma_start(b1row_sb[:], b1row[:])
            if include_b2:
                nc.vector.memset(ones16_sb[:], 1.0)
                nc.sync.dma_start(b2row_sb[:], b2row[:])

            pools = (sb, gpool, hpool, htpool, opool, wpool, fpool, wsump, psum_s, psum_g, psum_o)
            aps = (xdT, xsT, featT, xd_pc, xs_pc, w1T_sb, w2T_sb, b1row_sb,
                   b2row_sb, ones_sb, ones16_sb, identq_sb, out_ap, g_dram)
            fronts = []
            for b in range(BPC):
                fronts.append(_emit_front(nc, tc, pools, aps, b,
                                          include_b1, include_b2))
            for b in range(BPC):
                _emit_back(nc, tc, pools, aps, b, include_b1, include_b2,
                           fronts[b])

    nc.compile()
    return nc


_CACHE = {}


def _get_module(include_b1, include_b2):
    key = (include_b1, include_b2, tuple(sorted(_ABLATE)))
    if key not in _CACHE:
        _CACHE[key] = _build(include_b1, include_b2)
    return _CACHE[key]


def make_in_maps(xyz_dense, xyz_sparse, feat_sparse, W1, b1, W2, b2):
    xd = np.asarray(xyz_dense, np.float32)
    xs = np.asarray(xyz_sparse, np.float32)
    feat = np.asarray(feat_sparse, np.float32)

    # augmented score factors: S = 2 xd.xs - |xd|^2 - |xs|^2 = -d2
    xdT = np.empty((B, 5, N2), np.float32)
    xdT[:, 0:3] = 2.0 * xd.transpose(0, 2, 1)
    xdT[:, 3] = np.sum(xd * xd, -1)
    xdT[:, 4] = 1.0
    xsT = np.empty((B, 5, N1), np.float32)
    xsT[:, 0:3] = xs.transpose(0, 2, 1)
    xsT[:, 3] = -1.0
    xsT[:, 4] = -np.sum(xs * xs, -1)

    featT = np.ascontiguousarray(feat.transpose(0, 2, 1))
    # partition-major coords: [p, chunk, 3] with point index = chunk*128 + p
    xd_pc = np.ascontiguousarray(xd.reshape(B, NCH, 128, 3).transpose(0, 2, 1, 3))
    xs_pc = np.ascontiguousarray(xs.reshape(B, NR, 128, 3).transpose(0, 2, 1, 3))
    w1T = np.ascontiguousarray(np.asarray(W1, np.float32).T)
    w2T = np.ascontiguousarray(np.asarray(W2, np.float32).T.astype(np.float16))
    b1row = np.asarray(b1, np.float32).reshape(1, C_OUT)
    ident = np.arange(N2, dtype=np.int16)
    identq = np.zeros((128, N2 // 16), np.int16)
    for r in range(8):
        identq[16 * r:16 * (r + 1), :] = ident.reshape(N2 // 16, 16).T
    b2row = np.asarray(b2, np.float32).astype(np.float16).reshape(1, C_OUT)

    in_maps = []
    for core in range(N_CORES):
        s = slice(core * BPC, (core + 1) * BPC)
        in_maps.append({
            "xdT": np.ascontiguousarray(xdT[s]),
            "xsT": np.ascontiguousarray(xsT[s]),
            "featT": np.ascontiguousarray(featT[s]),
            "xd_pc": np.ascontiguousarray(xd_pc[s]),
            "xs_pc": np.ascontiguousarray(xs_pc[s]),
            "w1T": w1T, "w2T": w2T, "b1row": b1row, "b2row": b2row,
            "identq": identq,
        })
    return in_maps


def kernel(xyz_dense, xyz_sparse, feat_sparse, W1, b1, W2, b2):
    include_b1 = bool(np.any(np.asarray(b1) != 0))
    include_b2 = bool(np.any(np.asarray(b2) != 0))
    nc = _get_module(include_b1, include_b2)
    in_maps = make_in_maps(xyz_dense, xyz_sparse, feat_sparse, W1, b1, W2, b2)
    res = run_bass_kernel_spmd(nc, in_maps, list(range(N_CORES)))
    out = np.concatenate([res.results[i]["out"] for i in range(N_CORES)], axis=0)
    return np.ascontiguousarray(out.astype(np.float32))



# revision 14
# speedup vs baseline: 2.2632x; 2.2632x over previous
"""PointNet feature-propagation block on 8 Trainium2 NeuronCores.

Data-parallel over the batch dim: 16 batches -> 2 per core.
Per batch on-device pipeline:
  1. G = feat @ W1^T (+b1) per sparse point (PE, fp16 in / fp32 acc); rows
     [512 x fp16 | xs coords as 3 x fp32 | pad] (1280 B) stored to HBM.
  2. Scores S = -d2 via augmented K=13 split-fp16 matmul (PE):
     lhsT rows [2dh(3), 2dh(3), 2dl(3), ndh, ndl, 1, 1],
     rhs  rows [sh(3),  sl(3),  sh(3),  -1,  -1, -nsh, -nsl]
     (hi/lo fp16 splits recover fp32-grade |error| ~1e-6 on d2).
     Top-8 candidates per dense point: DVE max + max_index; keep 4.
  3-8 run as a per-8-chunk-group software pipeline chasing the scans:
  3. Wrapped int16 index lists for the group via a DRAM bounce.
  4. dma_gather of the 4 candidate G-rows per dense point.
  5. Exact d2 from gathered coords; worst-of-4 excluded by zero weight;
     u_m = keep_m/(sqrt(d2_m)+eps).
  6. h = relu(sf * sum_m diag(u_m) @ g_m): DVE builds diagonal weight tiles,
     PE accumulates the weighted rows in PSUM, ACT fuses 1/sum scale + relu.
  7. h^T via XBAR DMA transposes (HWDGE, ACT queue).
  8. out = h @ W2^T (+b2) (PE), stored as (dense, ch) fp16, widened on host.
"""
import numpy as np

import concourse.bass as bass
import concourse.tile as tile
import concourse.mybir as mybir
from concourse import bacc
from concourse.bass_utils import run_bass_kernel_spmd

B, N1, N2 = 16, 1024, 4096
C_IN, C_OUT = 512, 512
NCAND = 4                    # candidates gathered per dense point
KS = 13                      # score-matmul contraction rows
EPS = 1e-10
N_CORES = 8
BPC = B // N_CORES           # batches per core
NCH = N2 // 128              # dense chunks per batch (32)
NR = N1 // 128               # sparse chunks (8)
NQI = C_IN // 128            # input-channel chunks (4)
NQO = C_OUT // 128           # output-channel chunks (4)
JQ = 8                       # dense chunks per pipeline group
NGRP = NCH // JQ             # 4 groups per batch
GROW = 640                   # G row length in fp16 elems (512 feat + 6 coord + pad)

F32 = mybir.dt.float32
F16 = mybir.dt.float16
U16 = mybir.dt.uint16
I16 = mybir.dt.int16
Alu = mybir.AluOpType
Act = mybir.ActivationFunctionType
AxX = mybir.AxisListType.X


class _NS:
    pass


def _front_loads(nc, P, b):
    st = _NS()
    st.b = b
    st.xdT_sb = P.sb.tile([KS, N2], F16, tag="xdT")
    nc.sync.dma_start(st.xdT_sb[:], P.xdT[b])
    st.xsT_sb = P.sb.tile([KS, N1], F16, tag="xsT")
    nc.sync.dma_start(st.xsT_sb[:], P.xsT[b])
    st.xd_pc_sb = P.fpool.tile([128, NCH, 3], F32, tag="xd_pc")
    nc.sync.dma_start(st.xd_pc_sb[:], P.xd_pc[b])
    st.featT_sb = P.sb.tile([128, NQI, N1], F16, tag="featT")
    nc.sync.dma_start(st.featT_sb[:],
                      P.featT[b].rearrange("(q p) n -> p q n", p=128))
    st.xs_pc_sb = P.sb.tile([128, NR, 3], F32, tag="xs_pc")
    nc.sync.dma_start(st.xs_pc_sb[:], P.xs_pc[b])
    st.m8 = P.fpool.tile([128, NCH, 8], F16, tag="m8")
    st.i8 = P.fpool.tile([128, NCH, 8], U16, tag="i8")
    st.idxw = P.fpool.tile([128, NCAND, N2 // 16], I16, tag="idxw")
    return st


def _l1_stage(nc, P, st, include_b1):
    b = st.b
    g_sb = P.sb.tile([128, NR, GROW], F16, tag="g_sb")
    for r in range(NR):
        pg = P.psum_a.tile([128, C_OUT], F32, tag="acc")
        for q in range(NQI):
            nc.tensor.matmul(
                pg[:], st.featT_sb[:, q, r * 128:(r + 1) * 128], P.w1T_sb[:, q, :],
                start=(q == 0), stop=(q == NQI - 1 and not include_b1))
        if include_b1:
            nc.tensor.matmul(pg[:], P.ones16_sb[0:1, :], P.b1row_sb[:],
                             start=False, stop=True)
        nc.scalar.activation(g_sb[:, r, 0:C_OUT], pg[:], Act.Copy)
    nc.vector.tensor_copy(g_sb[:, :, C_OUT:C_OUT + 6].bitcast(F32), st.xs_pc_sb[:])
    nc.vector.memset(g_sb[:, :, C_OUT + 6:GROW], 0.0)
    # store to HBM with row-major (1024, GROW) layout: row r*128+p
    nc.sync.dma_start(P.g_dram[b].rearrange("(r p) e -> p r e", p=128), g_sb[:])


def _score_group(nc, P, st, Q):
    """Score matmuls for 8 chunks -> PSUM -> ACT copy to SBUF fp16 (this
    decouples the PE score matmuls from the slow DVE scans), then scans."""
    scs = P.spool.tile([128, JQ, N1], F16, tag="scs")
    for k in range(JQ):
        c = Q * JQ + k
        ps = P.psum_s.tile([128, N1], F32)
        lhs = st.xdT_sb[:, c * 128:(c + 1) * 128]
        nc.tensor.matmul(ps[:, 0:512], lhs, st.xsT_sb[:, 0:512], start=True, stop=True)
        nc.tensor.matmul(ps[:, 512:1024], lhs, st.xsT_sb[:, 512:1024], start=True, stop=True)
        nc.scalar.activation(scs[:, k, :], ps[:], Act.Copy)
    for k in range(JQ):
        c = Q * JQ + k
        nc.vector.max(st.m8[:, c, :], scs[:, k, :])
        nc.vector.max_index(st.i8[:, c, :], st.m8[:, c, :], scs[:, k, :])


def _idx_group(nc, P, st, Q):
    """Bounce this group's top-4 indices through DRAM into the wrapped
    int16 SWDGE layout: dst[q, m, c*8+pp] = i8[pp*16+q, c, m]."""
    b = st.b
    cs = slice(Q * JQ, (Q + 1) * JQ)
    nc.sync.dma_start(P.i8_dram[b, Q], st.i8[:, cs, 0:NCAND].bitcast(I16))
    wcol = slice(Q * JQ * 8, (Q + 1) * JQ * 8)
    for m in range(NCAND):
        nc.sync.dma_start(
            st.idxw[0:16, m, wcol].rearrange("p (c pp) -> p c pp", pp=8),
            P.i8_dram[b, Q][:, :, m].rearrange("(pp q) c -> q c pp", pp=8))
    for r in (16, 32, 64):
        nc.sync.dma_start(st.idxw[r:2 * r, :, wcol], st.idxw[0:r, :, wcol])


def _gather_group(nc, P, st, Q):
    b = st.b
    st.gk = getattr(st, "gk", {})
    gk = []
    for m in range(NCAND):
        t = P.gpool.tile([128, JQ, GROW], F16, tag=f"gk{m}")
        nc.gpsimd.dma_gather(
            out_ap=t[:], in_ap=P.g_dram[b],
            idxs_ap=st.idxw[:, m, Q * (JQ * 128 // 16):(Q + 1) * (JQ * 128 // 16)],
            num_idxs=JQ * 128, num_idxs_reg=JQ * 128, elem_size=GROW,
            queue_num=m)
        gk.append(t)
    st.gk[Q] = gk


def _back_group(nc, P, st, Q, include_b2):
    b = st.b
    js = slice(Q * JQ, (Q + 1) * JQ)
    gk = st.gk.pop(Q)

    # exact d2 for each candidate from gathered coords
    d2q = P.wpool.tile([128, JQ, NCAND], F32, tag="d2q")
    for m in range(NCAND):
        cview = gk[m][:, :, C_OUT:C_OUT + 6].bitcast(F32)
        diff = P.wpool.tile([128, JQ, 3], F32, tag="cdiff")
        nc.vector.scalar_tensor_tensor(
            diff[:], cview, -1.0, st.xd_pc_sb[:, js, :], op0=Alu.mult, op1=Alu.add)
        sq = P.wpool.tile([128, JQ, 3], F32, tag="csq")
        nc.vector.tensor_tensor(sq[:], diff[:], diff[:], Alu.mult)
        nc.vector.tensor_reduce(d2q[:, :, m:m + 1], sq[:], AxX, Alu.add)

    # weights: u = keep/(sqrt(d2)+eps); worst-of-4 excluded; the 1/sum
    # normalization rides the final ACT relu as a per-partition scale
    dmax = P.wpool.tile([128, JQ, 1], F32, tag="dmax")
    nc.vector.tensor_reduce(dmax[:], d2q[:], AxX, Alu.max)
    keep = P.wpool.tile([128, JQ, NCAND], F32, tag="keep")
    nc.vector.tensor_tensor(keep[:], d2q[:],
                            dmax[:].broadcast_to([128, JQ, NCAND]), Alu.is_lt)
    dist = P.wpool.tile([128, JQ, NCAND], F32, tag="cdist")
    nc.scalar.activation(dist[:], d2q[:], Act.Sqrt, bias=EPS * EPS)
    u = P.wpool.tile([128, JQ, NCAND], F32, tag="cu")
    nc.vector.reciprocal(u[:], dist[:])
    u0 = P.wpool.tile([128, JQ, NCAND], F32, tag="cu0")
    nc.vector.tensor_tensor(u0[:], u[:], keep[:], Alu.mult)
    usum = P.wpool.tile([128, JQ, 1], F32, tag="cusum")
    nc.vector.tensor_reduce(usum[:], u0[:], AxX, Alu.add)
    sf = P.wpool.tile([128, JQ, 1], F32, tag="csf")
    nc.vector.reciprocal(sf[:], usum[:])

    h_sb = P.hpool.tile([128, JQ, C_OUT], F16, tag="h_sb")
    for jj in range(JQ):
        # h = relu(sf * sum_m diag(u_m) @ g_m): DVE builds the diagonal
        # weight tiles (cheap 128-wide ops), PE accumulates the weighted
        # rows in PSUM, ACT applies the 1/sum scale + relu in one pass.
        ph = P.psum_a.tile([128, C_OUT], F32, tag="acc")
        for m in range(NCAND):
            dg = P.wsump.tile([128, 128], F16, tag="diag")
            nc.vector.tensor_scalar(dg[:], P.ident_sb[:],
                                    scalar1=u0[:, jj, m:m + 1], op0=Alu.mult,
                                    scalar2=1.0, op1=Alu.mult)
            nc.tensor.matmul(ph[:], dg[:], gk[m][:, jj, 0:C_OUT],
                             start=(m == 0), stop=(m == NCAND - 1))
        nc.scalar.activation(h_sb[:, jj, :], ph[:], Act.Relu,
                             scale=sf[:, jj, 0:1])

    # transposes into a double-buffered hT; layer 2 runs one group later
    hT = P.htpool.tile([128, NQO, JQ * 128], F16, tag="hT")
    for k in range(JQ):
        nc.scalar.dma_start_transpose(hT[:, :, k * 128:(k + 1) * 128],
                                      h_sb[:, k, :])
    st.hT_q = getattr(st, "hT_q", {})
    st.hT_q[Q] = hT


def _l2_group(nc, P, st, Q, include_b2):
    b = st.b
    hT = st.hT_q.pop(Q)
    osb = P.opool.tile([128, JQ, C_OUT], F16, tag="osb")
    for k in range(JQ):
        po = P.psum_o.tile([128, C_OUT], F32)
        for q in range(NQO):
            nc.tensor.matmul(
                po[:], hT[:, q, k * 128:(k + 1) * 128], P.w2T_sb[:, q, :],
                start=(q == 0), stop=(q == NQO - 1 and not include_b2))
        if include_b2:
            nc.tensor.matmul(po[:], P.ones16_sb[0:1, :], P.b2row_sb[:],
                             start=False, stop=True)
        nc.scalar.activation(osb[:, k, :], po[:], Act.Copy)
    nc.sync.dma_start(
        P.out_ap[b, Q * JQ * 128:(Q + 1) * JQ * 128, :]
        .rearrange("(c p) e -> p c e", p=128),
        osb[:])


def _build(include_b1, include_b2):
    nc = bacc.Bacc("TRN2", target_bir_lowering=False, debug=False,
                   num_devices=N_CORES, num_swdge_queues=4)

    # register EPS^2 as a const AP so it can be an ACT Sqrt bias
    # (sqrt(d2 + EPS^2) == sqrt(d2) + EPS at fp32 for all reachable d2)
    _ct = nc.alloc_sbuf_tensor("const-float32-epssq", [128, 1], F32)
    nc.gpsimd.memset(_ct.ap(), EPS * EPS)
    nc.const_aps.aps[(F32, EPS * EPS)] = _ct.ap()
    nc.all_engine_barrier()

    P = _NS()
    P.xdT = nc.dram_tensor("xdT", [BPC, KS, N2], F16, kind="ExternalInput").ap()
    P.xsT = nc.dram_tensor("xsT", [BPC, KS, N1], F16, kind="ExternalInput").ap()
    P.featT = nc.dram_tensor("featT", [BPC, C_IN, N1], F16, kind="ExternalInput").ap()
    P.xd_pc = nc.dram_tensor("xd_pc", [BPC, 128, NCH, 3], F32, kind="ExternalInput").ap()
    P.xs_pc = nc.dram_tensor("xs_pc", [BPC, 128, NR, 3], F32, kind="ExternalInput").ap()
    w1T = nc.dram_tensor("w1T", [C_IN, C_OUT], F16, kind="ExternalInput").ap()
    w2T = nc.dram_tensor("w2T", [C_IN, C_OUT], F16, kind="ExternalInput").ap()
    b1row = nc.dram_tensor("b1row", [1, C_OUT], F16, kind="ExternalInput").ap()
    b2row = nc.dram_tensor("b2row", [1, C_OUT], F16, kind="ExternalInput").ap()
    ident = nc.dram_tensor("ident", [128, 128], F16, kind="ExternalInput").ap()
    P.out_ap = nc.dram_tensor("out", [BPC, N2, C_OUT], F16, kind="ExternalOutput").ap()
    P.g_dram = nc.dram_tensor("gscratch", [BPC, N1, GROW], F16).ap()
    P.i8_dram = nc.dram_tensor("i8scratch", [BPC, NGRP, 128, JQ, NCAND], I16).ap()

    with tile.TileContext(nc) as tc:
        with (
            tc.tile_pool(name="sb", bufs=1) as sb,
            tc.tile_pool(name="gpool", bufs=2) as gpool,
            tc.tile_pool(name="spool", bufs=2) as spool,
            tc.tile_pool(name="hpool", bufs=1) as hpool,
            tc.tile_pool(name="htpool", bufs=2) as htpool,
            tc.tile_pool(name="opool", bufs=1) as opool,
            tc.tile_pool(name="wpool", bufs=3) as wpool,
            tc.tile_pool(name="fpool", bufs=2) as fpool,
            tc.tile_pool(name="wsump", bufs=8) as wsump,
            tc.tile_pool(name="const", bufs=1) as constp,
            tc.tile_pool(name="psum_s", bufs=2, space="PSUM") as psum_s,
            tc.tile_pool(name="psum_a", bufs=2, space="PSUM") as psum_a,
            tc.tile_pool(name="psum_o", bufs=2, space="PSUM") as psum_o,
        ):
            P.sb, P.gpool, P.hpool, P.htpool, P.opool = sb, gpool, hpool, htpool, opool
            P.spool = spool
            P.wpool, P.fpool, P.wsump = wpool, fpool, wsump
            P.psum_s, P.psum_a, P.psum_o = psum_s, psum_a, psum_o

            P.w1T_sb = constp.tile([128, NQI, C_OUT], F16, tag="w1T")
            for q in range(NQI):
                nc.sync.dma_start(P.w1T_sb[:, q, :], w1T[q * 128:(q + 1) * 128, :])
            P.w2T_sb = constp.tile([128, NQI, C_OUT], F16, tag="w2T")
            for q in range(NQI):
                nc.sync.dma_start(P.w2T_sb[:, q, :], w2T[q * 128:(q + 1) * 128, :])
            P.b1row_sb = constp.tile([1, C_OUT], F16, tag="b1row")
            P.b2row_sb = constp.tile([1, C_OUT], F16, tag="b2row")
            P.ones16_sb = constp.tile([1, 128], F16, tag="ones16")
            P.ident_sb = constp.tile([128, 128], F16, tag="ident")
            nc.sync.dma_start(P.ident_sb[:], ident[:])
            if include_b1 or include_b2:
                nc.vector.memset(P.ones16_sb[:], 1.0)
            if include_b1:
                nc.sync.dma_start(P.b1row_sb[:], b1row[:])
            if include_b2:
                nc.sync.dma_start(P.b2row_sb[:], b2row[:])

            # global software pipeline over BPC*NGRP groups with a one-
            # group skew: iteration g emits the back stages of group g-1
            # first, then scores/scans/idx/gathers of group g
            sts = [None] * BPC
            NG = BPC * NGRP
            sts[0] = _front_loads(nc, P, 0)
            for g in range(NG + 2):
                if g > 0 and g - 1 < NG:
                    pb, pQ = divmod(g - 1, NGRP)
                    _back_group(nc, P, sts[pb], pQ, include_b2)
                if g > 1:
                    pb, pQ = divmod(g - 2, NGRP)
                    _l2_group(nc, P, sts[pb], pQ, include_b2)
                if g < NG:
                    b, Q = divmod(g, NGRP)
                    _score_group(nc, P, sts[b], Q)
                    _idx_group(nc, P, sts[b], Q)
                    if Q == 0:
                        _l1_stage(nc, P, sts[b], include_b1)
                    _gather_group(nc, P, sts[b], Q)
                    if Q == NGRP - 1 and b + 1 < BPC:
                        sts[b + 1] = _front_loads(nc, P, b + 1)

    nc.compile()
    return nc


_CACHE = {}


def _get_module(include_b1, include_b2):
    key = (include_b1, include_b2)
    if key not in _CACHE:
        _CACHE[key] = _build(include_b1, include_b2)
    return _CACHE[key]


def make_in_maps(xyz_dense, xyz_sparse, feat_sparse, W1, b1, W2, b2):
    xd = np.asarray(xyz_dense, np.float32)
    xs = np.asarray(xyz_sparse, np.float32)
    feat = np.asarray(feat_sparse, np.float32)
    f16 = np.float16

    # split-fp16 augmented score factors: S = 2 xd.xs - |xd|^2 - |xs|^2 = -d2
    dh = f16(xd); dl = f16(xd - dh.astype(np.float32))
    sh = f16(xs); sl = f16(xs - sh.astype(np.float32))
    nd = np.sum(xd * xd, -1); ns = np.sum(xs * xs, -1)
    ndh = f16(nd); ndl = f16(nd - ndh.astype(np.float32))
    nsh = f16(ns); nsl = f16(ns - nsh.astype(np.float32))

    xdT = np.empty((B, KS, N2), f16)
    xdT[:, 0:3] = (2.0 * dh.astype(np.float32)).astype(f16).transpose(0, 2, 1)
    xdT[:, 3:6] = xdT[:, 0:3]
    xdT[:, 6:9] = (2.0 * dl.astype(np.float32)).astype(f16).transpose(0, 2, 1)
    xdT[:, 9] = ndh
    xdT[:, 10] = ndl
    xdT[:, 11] = 1.0
    xdT[:, 12] = 1.0
    xsT = np.empty((B, KS, N1), f16)
    xsT[:, 0:3] = sh.transpose(0, 2, 1)
    xsT[:, 3:6] = sl.transpose(0, 2, 1)
    xsT[:, 6:9] = sh.transpose(0, 2, 1)
    xsT[:, 9] = -1.0
    xsT[:, 10] = -1.0
    xsT[:, 11] = -nsh
    xsT[:, 12] = -nsl

    featT = np.ascontiguousarray(feat.transpose(0, 2, 1).astype(f16))
    # partition-major coords: [p, chunk, 3] with point index = chunk*128 + p
    xd_pc = np.ascontiguousarray(xd.reshape(B, NCH, 128, 3).transpose(0, 2, 1, 3))
    xs_pc = np.ascontiguousarray(xs.reshape(B, NR, 128, 3).transpose(0, 2, 1, 3))
    w1T = np.ascontiguousarray(np.asarray(W1, np.float32).T.astype(f16))
    w2T = np.ascontiguousarray(np.asarray(W2, np.float32).T.astype(f16))
    b1row = np.asarray(b1, np.float32).astype(f16).reshape(1, C_OUT)
    b2row = np.asarray(b2, np.float32).astype(f16).reshape(1, C_OUT)
    ident16 = np.eye(128, dtype=f16)

    in_maps = []
    for core in range(N_CORES):
        s = slice(core * BPC, (core + 1) * BPC)
        in_maps.append({
            "xdT": np.ascontiguousarray(xdT[s]),
            "xsT": np.ascontiguousarray(xsT[s]),
            "featT": np.ascontiguousarray(featT[s]),
            "xd_pc": np.ascontiguousarray(xd_pc[s]),
            "xs_pc": np.ascontiguousarray(xs_pc[s]),
            "w1T": w1T, "w2T": w2T, "b1row": b1row, "b2row": b2row,
            "ident": ident16,
        })
    return in_maps


def kernel(xyz_dense, xyz_sparse, feat_sparse, W1, b1, W2, b2):
    include_b1 = bool(np.any(np.asarray(b1) != 0))
    include_b2 = bool(np.any(np.asarray(b2) != 0))
    nc = _get_module(include_b1, include_b2)
    in_maps = make_in_maps(xyz_dense, xyz_sparse, feat_sparse, W1, b1, W2, b2)
    res = run_bass_kernel_spmd(nc, in_maps, list(range(N_CORES)))
    out = np.concatenate([res.results[i]["out"] for i in range(N_CORES)], axis=0)
    return np.ascontiguousarray(out.astype(np.float32))


# revision 18
# speedup vs baseline: 3.8813x; 1.7150x over previous
"""PointNet feature-propagation block on 8 Trainium2 NeuronCores.

Data-parallel over the batch dim: 16 batches -> 2 per core.
Per batch on-device pipeline:
  1. G = feat @ W1^T (+b1) per sparse point (PE, fp16 in / fp32 acc); rows
     [512 x fp16 | xs coords as 3 x fp32 | pad] (1280 B) stored to HBM.
  2. Scores S = -d2 via augmented K=13 split-fp16 matmul (PE):
     lhsT rows [2dh(3), 2dh(3), 2dl(3), ndh, ndl, 1, 1],
     rhs  rows [sh(3),  sl(3),  sh(3),  -1,  -1, -nsh, -nsl]
     (hi/lo fp16 splits recover fp32-grade |error| ~1e-6 on d2).
     Top-8 candidates per dense point: DVE max + max_index; keep 4.
  3-8 run as a per-8-chunk-group software pipeline chasing the scans:
  3. Wrapped int16 index lists for the group via a DRAM bounce.
  4. dma_gather of the 4 candidate G-rows per dense point.
  5. Exact d2 from gathered coords; worst-of-4 excluded by zero weight;
     u_m = keep_m/(sqrt(d2_m)+eps).
  6. h = relu(sf * sum_m diag(u_m) @ g_m): DVE builds diagonal weight tiles,
     PE accumulates the weighted rows in PSUM, ACT fuses 1/sum scale + relu.
  7. h^T via XBAR DMA transposes (HWDGE, ACT queue).
  8. out = h @ W2^T (+b2) (PE), stored as (dense, ch) fp16, widened on host.
"""
import numpy as np

import concourse.bass as bass
import concourse.tile as tile
import concourse.mybir as mybir
from concourse import bacc
from concourse.bass_utils import run_bass_kernel_spmd

B, N1, N2 = 16, 1024, 4096
C_IN, C_OUT = 512, 512
NCAND = 4                    # candidates gathered per dense point
KS = 13                      # score-matmul contraction rows
EPS = 1e-10
N_CORES = 8
BPC = B // N_CORES           # batches per core
NCH = N2 // 128              # dense chunks per batch (32)
NR = N1 // 128               # sparse chunks (8)
NQI = C_IN // 128            # input-channel chunks (4)
NQO = C_OUT // 128           # output-channel chunks (4)
JQ = 4                       # dense chunks per pipeline group
NGRP = NCH // JQ             # 4 groups per batch
GROW = 640                   # G row length in fp16 elems (512 feat + 6 coord + pad)

F32 = mybir.dt.float32
F16 = mybir.dt.float16
U16 = mybir.dt.uint16
I16 = mybir.dt.int16
Alu = mybir.AluOpType
Act = mybir.ActivationFunctionType
AxX = mybir.AxisListType.X


class _NS:
    pass


def _front_loads(nc, P, b):
    st = _NS()
    st.sb_idx = b % (2 * BPC)    # DRAM scratch slot (rep-parity double buffer)
    st.b = b % BPC
    b = st.b
    st.xdT_sb = P.sb.tile([KS, N2], F16, tag="xdT")
    nc.sync.dma_start(st.xdT_sb[:], P.xdT[b])
    st.xsT_sb = P.sb.tile([KS, N1], F16, tag="xsT")
    nc.sync.dma_start(st.xsT_sb[:], P.xsT[b])
    st.xd_pc_sb = P.fpool.tile([128, NCH, 3], F32, tag="xd_pc")
    nc.sync.dma_start(st.xd_pc_sb[:], P.xd_pc[b])
    st.featT_sb = P.sb.tile([128, NQI, N1], F16, tag="featT")
    nc.sync.dma_start(st.featT_sb[:],
                      P.featT[b].rearrange("(q p) n -> p q n", p=128))
    st.xs_pc_sb = P.sb.tile([128, NR, 3], F32, tag="xs_pc")
    nc.sync.dma_start(st.xs_pc_sb[:], P.xs_pc[b])
    st.m8 = P.fpool.tile([128, NCH, 8], F16, tag="m8")
    st.i8 = P.fpool.tile([128, NCH, 8], U16, tag="i8")
    st.idxw = P.fpool.tile([128, NCAND, N2 // 16], I16, tag="idxw")
    return st


def _l1_stage(nc, P, st, include_b1):
    b = st.b
    g_sb = P.sb.tile([128, NR, GROW], F16, tag="g_sb")
    for r in range(NR):
        pg = P.psum_a.tile([128, C_OUT], F32, tag="acc")
        for q in range(NQI):
            nc.tensor.matmul(
                pg[:], st.featT_sb[:, q, r * 128:(r + 1) * 128], P.w1T_sb[:, q, :],
                start=(q == 0), stop=(q == NQI - 1 and not include_b1))
        if include_b1:
            nc.tensor.matmul(pg[:], P.ones16_sb[0:1, :], P.b1row_sb[:],
                             start=False, stop=True)
        nc.scalar.activation(g_sb[:, r, 0:C_OUT], pg[:], Act.Copy)
    nc.vector.tensor_copy(g_sb[:, :, C_OUT:C_OUT + 6].bitcast(F32), st.xs_pc_sb[:])
    nc.vector.memset(g_sb[:, :, C_OUT + 6:GROW], 0.0)
    # store to HBM with row-major (1024, GROW) layout: row r*128+p
    nc.sync.dma_start(P.g_dram[st.sb_idx].rearrange("(r p) e -> p r e", p=128), g_sb[:])


def _score_group(nc, P, st, Q):
    """Score matmuls for 8 chunks -> PSUM -> ACT copy to SBUF fp16 (this
    decouples the PE score matmuls from the slow DVE scans), then scans."""
    scs = P.spool.tile([128, JQ, N1], F16, tag="scs")
    for k in range(JQ):
        c = Q * JQ + k
        ps = P.psum_s.tile([128, N1], F32)
        lhs = st.xdT_sb[:, c * 128:(c + 1) * 128]
        nc.tensor.matmul(ps[:, 0:512], lhs, st.xsT_sb[:, 0:512], start=True, stop=True)
        nc.tensor.matmul(ps[:, 512:1024], lhs, st.xsT_sb[:, 512:1024], start=True, stop=True)
        nc.scalar.activation(scs[:, k, :], ps[:], Act.Copy)
    for k in range(JQ):
        c = Q * JQ + k
        nc.vector.max(st.m8[:, c, :], scs[:, k, :])
        nc.vector.max_index(st.i8[:, c, :], st.m8[:, c, :], scs[:, k, :])


def _idx_group(nc, P, st, Q):
    """Bounce this group's top-4 indices through DRAM into the wrapped
    int16 SWDGE layout: dst[q, m, c*8+pp] = i8[pp*16+q, c, m]."""
    b = st.b
    cs = slice(Q * JQ, (Q + 1) * JQ)
    nc.sync.dma_start(P.i8_dram[st.sb_idx, Q], st.i8[:, cs, 0:NCAND].bitcast(I16))
    wcol = slice(Q * JQ * 8, (Q + 1) * JQ * 8)
    for m in range(NCAND):
        nc.sync.dma_start(
            st.idxw[0:16, m, wcol].rearrange("p (c pp) -> p c pp", pp=8),
            P.i8_dram[st.sb_idx, Q][:, :, m].rearrange("(pp q) c -> q c pp", pp=8))
    for r in (16, 32, 64):
        nc.sync.dma_start(st.idxw[r:2 * r, :, wcol], st.idxw[0:r, :, wcol])


def _gather_group(nc, P, st, Q):
    b = st.b
    st.gk = getattr(st, "gk", {})
    gk = []
    for m in range(NCAND):
        t = P.gpool.tile([128, JQ, GROW], F16, tag=f"gk{m}")
        nc.gpsimd.dma_gather(
            out_ap=t[:], in_ap=P.g_dram[st.sb_idx],
            idxs_ap=st.idxw[:, m, Q * (JQ * 128 // 16):(Q + 1) * (JQ * 128 // 16)],
            num_idxs=JQ * 128, num_idxs_reg=JQ * 128, elem_size=GROW,
            queue_num=m)
        gk.append(t)
    st.gk[Q] = gk


def _weights_group(nc, P, st, Q):
    """Exact d2 from gathered coords + inverse-distance weights; emitted
    right after the group's gathers so the next iteration's wsum matmuls
    find u0/sf ready."""
    b = st.b
    js = slice(Q * JQ, (Q + 1) * JQ)
    gk = st.gk[Q]
    d2q = P.wpool.tile([128, JQ, NCAND], F32, tag="d2q")
    for m in range(NCAND):
        cview = gk[m][:, :, C_OUT:C_OUT + 6].bitcast(F32)
        diff = P.wpool.tile([128, JQ, 3], F32, tag="cdiff")
        nc.vector.scalar_tensor_tensor(
            diff[:], cview, -1.0, st.xd_pc_sb[:, js, :], op0=Alu.mult, op1=Alu.add)
        sq = P.wpool.tile([128, JQ, 3], F32, tag="csq")
        nc.vector.tensor_tensor(sq[:], diff[:], diff[:], Alu.mult)
        nc.vector.tensor_reduce(d2q[:, :, m:m + 1], sq[:], AxX, Alu.add)
    dmax = P.wpool.tile([128, JQ, 1], F32, tag="dmax")
    nc.vector.tensor_reduce(dmax[:], d2q[:], AxX, Alu.max)
    keep = P.wpool.tile([128, JQ, NCAND], F32, tag="keep")
    nc.vector.tensor_tensor(keep[:], d2q[:],
                            dmax[:].broadcast_to([128, JQ, NCAND]), Alu.is_lt)
    dist = P.wpool.tile([128, JQ, NCAND], F32, tag="cdist")
    nc.scalar.activation(dist[:], d2q[:], Act.Sqrt, bias=EPS * EPS)
    u = P.wpool.tile([128, JQ, NCAND], F32, tag="cu")
    nc.vector.reciprocal(u[:], dist[:])
    u0 = P.wpool.tile([128, JQ, NCAND], F32, tag="cu0")
    nc.vector.tensor_tensor(u0[:], u[:], keep[:], Alu.mult)
    usum = P.wpool.tile([128, JQ, 1], F32, tag="cusum")
    nc.vector.tensor_reduce(usum[:], u0[:], AxX, Alu.add)
    sf = P.wpool.tile([128, JQ, 1], F32, tag="csf")
    nc.vector.reciprocal(sf[:], usum[:])
    st.w_q = getattr(st, "w_q", {})
    st.w_q[Q] = (u0, sf)


def _back_group(nc, P, st, Q, include_b2):
    b = st.b
    gk = st.gk.pop(Q)
    u0, sf = st.w_q.pop(Q)
    h_sb = P.hpool.tile([128, JQ, C_OUT], F16, tag="h_sb")
    for jj in range(JQ):
        # h = relu(sf * sum_m diag(u_m) @ g_m): DVE builds the diagonal
        # weight tiles (cheap 128-wide ops), PE accumulates the weighted
        # rows in PSUM, ACT applies the 1/sum scale + relu in one pass.
        ph = P.psum_a.tile([128, C_OUT], F32, tag="acc")
        for m in range(NCAND):
            dg = P.wsump.tile([128, 128], F16, tag="diag")
            nc.vector.tensor_scalar(dg[:], P.ident_sb[:],
                                    scalar1=u0[:, jj, m:m + 1], op0=Alu.mult,
                                    scalar2=1.0, op1=Alu.mult)
            nc.tensor.matmul(ph[:], dg[:], gk[m][:, jj, 0:C_OUT],
                             start=(m == 0), stop=(m == NCAND - 1))
        nc.scalar.activation(h_sb[:, jj, :], ph[:], Act.Relu,
                             scale=sf[:, jj, 0:1])

    # transposes into a double-buffered hT; layer 2 runs one group later
    hT = P.htpool.tile([128, NQO, JQ * 128], F16, tag="hT")
    for k in range(JQ):
        nc.scalar.dma_start_transpose(hT[:, :, k * 128:(k + 1) * 128],
                                      h_sb[:, k, :])
    st.hT_q = getattr(st, "hT_q", {})
    st.hT_q[Q] = hT


def _l2_group(nc, P, st, Q, include_b2):
    b = st.b
    hT = st.hT_q.pop(Q)
    osb = P.opool.tile([128, JQ, C_OUT], F16, tag="osb")
    for k in range(JQ):
        po = P.psum_o.tile([128, C_OUT], F32)
        for q in range(NQO):
            nc.tensor.matmul(
                po[:], hT[:, q, k * 128:(k + 1) * 128], P.w2T_sb[:, q, :],
                start=(q == 0), stop=(q == NQO - 1 and not include_b2))
        if include_b2:
            nc.tensor.matmul(po[:], P.ones16_sb[0:1, :], P.b2row_sb[:],
                             start=False, stop=True)
        nc.scalar.activation(osb[:, k, :], po[:], Act.Copy)
    nc.sync.dma_start(
        P.out_ap[b, Q * JQ * 128:(Q + 1) * JQ * 128, :]
        .rearrange("(c p) e -> p c e", p=128),
        osb[:])


def _build(include_b1, include_b2, reps=1):
    nc = bacc.Bacc("TRN2", target_bir_lowering=False, debug=False,
                   num_devices=N_CORES, num_swdge_queues=4)

    # register EPS^2 as a const AP so it can be an ACT Sqrt bias
    # (sqrt(d2 + EPS^2) == sqrt(d2) + EPS at fp32 for all reachable d2)
    _ct = nc.alloc_sbuf_tensor("const-float32-epssq", [128, 1], F32)
    nc.gpsimd.memset(_ct.ap(), EPS * EPS)
    nc.const_aps.aps[(F32, EPS * EPS)] = _ct.ap()
    nc.all_engine_barrier()

    P = _NS()
    P.xdT = nc.dram_tensor("xdT", [BPC, KS, N2], F16, kind="ExternalInput").ap()
    P.xsT = nc.dram_tensor("xsT", [BPC, KS, N1], F16, kind="ExternalInput").ap()
    P.featT = nc.dram_tensor("featT", [BPC, C_IN, N1], F16, kind="ExternalInput").ap()
    P.xd_pc = nc.dram_tensor("xd_pc", [BPC, 128, NCH, 3], F32, kind="ExternalInput").ap()
    P.xs_pc = nc.dram_tensor("xs_pc", [BPC, 128, NR, 3], F32, kind="ExternalInput").ap()
    w1T = nc.dram_tensor("w1T", [C_IN, C_OUT], F16, kind="ExternalInput").ap()
    w2T = nc.dram_tensor("w2T", [C_IN, C_OUT], F16, kind="ExternalInput").ap()
    b1row = nc.dram_tensor("b1row", [1, C_OUT], F16, kind="ExternalInput").ap()
    b2row = nc.dram_tensor("b2row", [1, C_OUT], F16, kind="ExternalInput").ap()
    ident = nc.dram_tensor("ident", [128, 128], F16, kind="ExternalInput").ap()
    P.out_ap = nc.dram_tensor("out", [BPC, N2, C_OUT], F16, kind="ExternalOutput").ap()
    P.g_dram = nc.dram_tensor("gscratch", [2 * BPC, N1, GROW], F16).ap()
    P.i8_dram = nc.dram_tensor("i8scratch", [2 * BPC, NGRP, 128, JQ, NCAND], I16).ap()

    with tile.TileContext(nc) as tc:
        with (
            tc.tile_pool(name="sb", bufs=1) as sb,
            tc.tile_pool(name="gpool", bufs=2) as gpool,
            tc.tile_pool(name="spool", bufs=2) as spool,
            tc.tile_pool(name="hpool", bufs=1) as hpool,
            tc.tile_pool(name="htpool", bufs=2) as htpool,
            tc.tile_pool(name="opool", bufs=1) as opool,
            tc.tile_pool(name="wpool", bufs=3) as wpool,
            tc.tile_pool(name="fpool", bufs=2) as fpool,
            tc.tile_pool(name="wsump", bufs=8) as wsump,
            tc.tile_pool(name="const", bufs=1) as constp,
            tc.tile_pool(name="psum_s", bufs=2, space="PSUM") as psum_s,
            tc.tile_pool(name="psum_a", bufs=2, space="PSUM") as psum_a,
            tc.tile_pool(name="psum_o", bufs=2, space="PSUM") as psum_o,
        ):
            P.sb, P.gpool, P.hpool, P.htpool, P.opool = sb, gpool, hpool, htpool, opool
            P.spool = spool
            P.wpool, P.fpool, P.wsump = wpool, fpool, wsump
            P.psum_s, P.psum_a, P.psum_o = psum_s, psum_a, psum_o

            P.w1T_sb = constp.tile([128, NQI, C_OUT], F16, tag="w1T")
            for q in range(NQI):
                nc.sync.dma_start(P.w1T_sb[:, q, :], w1T[q * 128:(q + 1) * 128, :])
            P.w2T_sb = constp.tile([128, NQI, C_OUT], F16, tag="w2T")
            for q in range(NQI):
                nc.sync.dma_start(P.w2T_sb[:, q, :], w2T[q * 128:(q + 1) * 128, :])
            P.b1row_sb = constp.tile([1, C_OUT], F16, tag="b1row")
            P.b2row_sb = constp.tile([1, C_OUT], F16, tag="b2row")
            P.ones16_sb = constp.tile([1, 128], F16, tag="ones16")
            P.ident_sb = constp.tile([128, 128], F16, tag="ident")
            nc.sync.dma_start(P.ident_sb[:], ident[:])
            if include_b1 or include_b2:
                nc.vector.memset(P.ones16_sb[:], 1.0)
            if include_b1:
                nc.sync.dma_start(P.b1row_sb[:], b1row[:])
            if include_b2:
                nc.sync.dma_start(P.b2row_sb[:], b2row[:])

            # global software pipeline over BPC*NGRP groups with a one-
            # group skew: iteration g emits the back stages of group g-1
            # first, then scores/scans/idx/gathers of group g
            sts = [None] * BPC
            NG = BPC * NGRP * reps
            NGB = BPC * reps
            sts = [None] * NGB
            sts[0] = _front_loads(nc, P, 0)
            for g in range(NG + 2):
                if g > 0 and g - 1 < NG:
                    pb, pQ = divmod(g - 1, NGRP)
                    _back_group(nc, P, sts[pb], pQ, include_b2)
                if g > 1:
                    pb, pQ = divmod(g - 2, NGRP)
                    _l2_group(nc, P, sts[pb], pQ, include_b2)
                if g < NG:
                    b, Q = divmod(g, NGRP)
                    _score_group(nc, P, sts[b], Q)
                    _idx_group(nc, P, sts[b], Q)
                    if Q == 0:
                        _l1_stage(nc, P, sts[b], include_b1)
                    _gather_group(nc, P, sts[b], Q)
                    _weights_group(nc, P, sts[b], Q)
                    if Q == NGRP - 1 and b + 1 < NGB:
                        sts[b + 1] = _front_loads(nc, P, b + 1)

    nc.compile()
    return nc


_CACHE = {}


def _get_module(include_b1, include_b2, reps=1):
    key = (include_b1, include_b2, reps)
    if key not in _CACHE:
        _CACHE[key] = _build(include_b1, include_b2, reps)
    return _CACHE[key]


def make_in_maps(xyz_dense, xyz_sparse, feat_sparse, W1, b1, W2, b2):
    xd = np.asarray(xyz_dense, np.float32)
    xs = np.asarray(xyz_sparse, np.float32)
    feat = np.asarray(feat_sparse, np.float32)
    f16 = np.float16

    # split-fp16 augmented score factors: S = 2 xd.xs - |xd|^2 - |xs|^2 = -d2
    dh = f16(xd); dl = f16(xd - dh.astype(np.float32))
    sh = f16(xs); sl = f16(xs - sh.astype(np.float32))
    nd = np.sum(xd * xd, -1); ns = np.sum(xs * xs, -1)
    ndh = f16(nd); ndl = f16(nd - ndh.astype(np.float32))
    nsh = f16(ns); nsl = f16(ns - nsh.astype(np.float32))

    xdT = np.empty((B, KS, N2), f16)
    xdT[:, 0:3] = (2.0 * dh.astype(np.float32)).astype(f16).transpose(0, 2, 1)
    xdT[:, 3:6] = xdT[:, 0:3]
    xdT[:, 6:9] = (2.0 * dl.astype(np.float32)).astype(f16).transpose(0, 2, 1)
    xdT[:, 9] = ndh
    xdT[:, 10] = ndl
    xdT[:, 11] = 1.0
    xdT[:, 12] = 1.0
    xsT = np.empty((B, KS, N1), f16)
    xsT[:, 0:3] = sh.transpose(0, 2, 1)
    xsT[:, 3:6] = sl.transpose(0, 2, 1)
    xsT[:, 6:9] = sh.transpose(0, 2, 1)
    xsT[:, 9] = -1.0
    xsT[:, 10] = -1.0
    xsT[:, 11] = -nsh
    xsT[:, 12] = -nsl

    featT = np.ascontiguousarray(feat.transpose(0, 2, 1).astype(f16))
    # partition-major coords: [p, chunk, 3] with point index = chunk*128 + p
    xd_pc = np.ascontiguousarray(xd.reshape(B, NCH, 128, 3).transpose(0, 2, 1, 3))
    xs_pc = np.ascontiguousarray(xs.reshape(B, NR, 128, 3).transpose(0, 2, 1, 3))
    w1T = np.ascontiguousarray(np.asarray(W1, np.float32).T.astype(f16))
    w2T = np.ascontiguousarray(np.asarray(W2, np.float32).T.astype(f16))
    b1row = np.asarray(b1, np.float32).astype(f16).reshape(1, C_OUT)
    b2row = np.asarray(b2, np.float32).astype(f16).reshape(1, C_OUT)
    ident16 = np.eye(128, dtype=f16)

    in_maps = []
    for core in range(N_CORES):
        s = slice(core * BPC, (core + 1) * BPC)
        in_maps.append({
            "xdT": np.ascontiguousarray(xdT[s]),
            "xsT": np.ascontiguousarray(xsT[s]),
            "featT": np.ascontiguousarray(featT[s]),
            "xd_pc": np.ascontiguousarray(xd_pc[s]),
            "xs_pc": np.ascontiguousarray(xs_pc[s]),
            "w1T": w1T, "w2T": w2T, "b1row": b1row, "b2row": b2row,
            "ident": ident16,
        })
    return in_maps


def kernel(xyz_dense, xyz_sparse, feat_sparse, W1, b1, W2, b2):
    include_b1 = bool(np.any(np.asarray(b1) != 0))
    include_b2 = bool(np.any(np.asarray(b2) != 0))
    nc = _get_module(include_b1, include_b2)
    in_maps = make_in_maps(xyz_dense, xyz_sparse, feat_sparse, W1, b1, W2, b2)
    res = run_bass_kernel_spmd(nc, in_maps, list(range(N_CORES)))
    out = np.concatenate([res.results[i]["out"] for i in range(N_CORES)], axis=0)
    return np.ascontiguousarray(out.astype(np.float32))
